# revision 2
# baseline (speedup 1.0000x reference)
"""Batch per-sample 3x3 conv (B=32, C=32, H=W=256, pad=1) on 8 TRN2 cores.

Data parallel: 4 samples per core, stacked on the 4 32-partition groups
(128 partitions = 4 samples x 32 channels), block-diagonal stationaries.

1D Winograd F(2,3) along x cuts the PE streaming work from 9 to 6
column-streams per output pixel: for each ky tap (dy in {-1,0,1}) the
3-tap x-conv of an output pair (o_even, o_odd) is computed from 4
Winograd products M0..M3, where V0..V3 are +-combinations of stride-2
input columns (computed on VectorE/GpSimd in fp16) and U0..U3 are
host-transformed weights. Per 4-output-row group, 12 accumulating
matmuls (4 m-values x 3 dy) of N=512 produce M-banks in PSUM; the
output transform o_even = M0+M1+M2+b, o_odd = M1-M2-M3+b runs as
2 ScalarE PSUM->SBUF copies, 2 GpSimd SBUF adds, and 2 VectorE
scalar_tensor_tensor ops (one PSUM operand each, bias fused), writing
fp16 outputs that are upcast on the host.
"""

import numpy as np

N_CORES = 8
B, C_IN, C_OUT, H, W, KS = 32, 32, 32, 256, 256, 3
SPC = B // N_CORES  # samples per core
CH = 32  # output rows per chunk
NCH = H // CH
NG = CH // 4  # 4-row groups per chunk
TPR = W // 2  # winograd tiles (output pairs) per row

_CACHE = {}


def _build():
    import concourse.bacc as bacc
    import concourse.mybir as mybir
    import concourse.tile as tile

    f32 = mybir.dt.float32
    f16 = mybir.dt.float16
    AL = mybir.AluOpType

    nc = bacc.Bacc(
        "TRN2", target_bir_lowering=False, debug=False, num_devices=N_CORES
    )
    x_d = nc.dram_tensor("x", [128, H, W], f16, kind="ExternalInput").ap()
    w_d = nc.dram_tensor("w", [128, 12 * 128], f16, kind="ExternalInput").ap()
    bias_d = nc.dram_tensor("bias_v", [128, 1], f32, kind="ExternalInput").ap()
    o_d = nc.dram_tensor("out", [128, H, W], f16, kind="ExternalOutput").ap()

    with tile.TileContext(nc) as tc:
        with (
            tc.tile_pool(name="const", bufs=1) as cpool,
            tc.tile_pool(name="xp", bufs=1) as xpool,
            tc.tile_pool(name="vp", bufs=1) as vpool,
            tc.tile_pool(name="dr", bufs=2) as dpool,
            tc.tile_pool(name="op", bufs=2) as opool,
            tc.tile_pool(name="ps", bufs=8, space="PSUM") as ppool,
        ):
            # weights/bias on the gpsimd queues so the sync-engine queues
            # are free for the first x pieces
            w_sb = cpool.tile([128, 12 * 128], f16)
            nc.gpsimd.dma_start(out=w_sb[:, 0:256], in_=w_d[:, 0:256])
            nc.gpsimd.dma_start(out=w_sb[:, 256:], in_=w_d[:, 256:])
            b_sb = cpool.tile([128, 1], f32)
            nc.gpsimd.dma_start(out=b_sb[:], in_=bias_d[:])

            # x row windows: [slot, col]; col 0 / 257 are permanent x-pads
            xbufs = [
                xpool.tile([128, CH + 2, W + 2], f16, tag=f"xb{i}", name=f"xb{i}")
                for i in range(2)
            ]
            for xb in xbufs:
                nc.vector.memset(xb[:, :, 0:1], 0)
                nc.vector.memset(xb[:, :, W + 1 : W + 2], 0)
            # top pad row (input row -1) for chunk 0
            nc.vector.memset(xbufs[0][:, 0:1, :], 0)

            vbufs = [
                vpool.tile(
                    [128, 4, CH + 2, TPR], f16, tag=f"vb{i}", name=f"vb{i}"
                )
                for i in range(2)
            ]

            # warm the PE clock (HAM gate) during the initial x DMA wait
            dumw = cpool.tile([128, 640], f16)
            nc.vector.memset(dumw[:], 0)
            psw = ppool.tile([128, 4, TPR], f32, tag="ps0", name="psw", bufs=1)
            NWARM = 16
            for k in range(NWARM):
                nc.tensor.matmul(
                    psw[:, :, :],
                    dumw[:, 0:128],
                    dumw[:, 128:640],
                    start=(k == 0),
                    stop=(k == NWARM - 1),
                )

            # V_m[t] taps: (colA, colB, op) on the padded row
            #   V0 = xp[2t]   - xp[2t+2]
            #   V1 = xp[2t+1] + xp[2t+2]
            #   V2 = xp[2t+2] - xp[2t+1]
            #   V3 = xp[2t+1] - xp[2t+3]
            VDEF = [
                (0, 2, AL.subtract),
                (1, 2, AL.add),
                (2, 1, AL.subtract),
                (1, 3, AL.subtract),
            ]

            for ch in range(NCH):
                r0 = ch * CH
                xb = xbufs[ch % 2]
                vb = vbufs[ch % 2]
                # input rows r0-1 .. r0+CH land on slots 0..CH+1
                lo = max(r0 - 1, 0)
                hi = min(r0 + CH + 1, H)
                dst0 = lo - (r0 - 1)
                if ch == 0:
                    bounds = [0, 4, 16, hi]
                else:
                    t = (hi - lo) // 3
                    bounds = [lo, lo + t, lo + 2 * t, hi]
                for a, b in zip(bounds[:-1], bounds[1:]):
                    nc.sync.dma_start(
                        out=xb[:, dst0 + (a - lo) : dst0 + (b - lo), 1 : W + 1],
                        in_=x_d[:, a:b, :],
                    )
                if hi < r0 + CH + 1:  # bottom pad row (input row H)
                    nc.vector.memset(xb[:, CH + 1 : CH + 2, :], 0)

                # V transform; m0/m1 on VectorE, m2/m3 on GpSimd
                vsplit = [(0, 10), (10, 22), (22, CH + 2)]
                for m, (ca, cb, op) in enumerate(VDEF):
                    eng = nc.vector if m < 2 else nc.gpsimd
                    for ra, rb in vsplit:
                        eng.tensor_tensor(
                            vb[:, m, ra:rb, :],
                            xb[:, ra:rb, ca : ca + 2 * TPR - 1 : 2],
                            xb[:, ra:rb, cb : cb + 2 * TPR - 1 : 2],
                            op,
                        )

                ob = opool.tile([128, CH, W], f16, tag="ob", name="ob", bufs=2)
                for g in range(NG):
                    pss = [
                        ppool.tile(
                            [128, 4, TPR],
                            f32,
                            tag=f"ps{(g % 2) * 4 + m}",
                            name=f"ps{(g % 2) * 4 + m}",
                            bufs=1,
                        )
                        for m in range(4)
                    ]
                    for m in range(4):
                        for dyi in range(3):
                            sl = 4 * g + dyi
                            nc.tensor.matmul(
                                pss[m][:, :, :],
                                w_sb[:, (3 * m + dyi) * 128 : (3 * m + dyi + 1) * 128],
                                vb[:, m, sl : sl + 4, :],
                                start=(dyi == 0),
                                stop=(dyi == 2),
                            )
                    # output transform: even = M0+M1+M2+b, odd = M1-M2-M3+b
                    # (M3 bank holds -M3 via negated U3)
                    c1 = dpool.tile([128, 4, TPR], f32, tag="c1", name="c1")
                    c2 = dpool.tile([128, 4, TPR], f32, tag="c2", name="c2")
                    tt = dpool.tile([128, 4, TPR], f32, tag="tt", name="tt")
                    uu = dpool.tile([128, 4, TPR], f32, tag="uu", name="uu")
                    nc.scalar.copy(out=c1[:, :, :], in_=pss[1][:, :, :])
                    nc.scalar.copy(out=c2[:, :, :], in_=pss[2][:, :, :])
                    nc.gpsimd.tensor_add(tt[:, :, :], c1[:, :, :], c2[:, :, :])
                    nc.gpsimd.tensor_sub(uu[:, :, :], c1[:, :, :], c2[:, :, :])
                    nc.vector.scalar_tensor_tensor(
                        out=ob[:, 4 * g : 4 * g + 4, 0 : W - 1 : 2],
                        in0=pss[0][:, :, :],
                        scalar=b_sb[:, :],
                        in1=tt[:, :, :],
                        op0=AL.add,
                        op1=AL.add,
                    )
                    nc.vector.scalar_tensor_tensor(
                        out=ob[:, 4 * g : 4 * g + 4, 1:W:2],
                        in0=pss[3][:, :, :],
                        scalar=b_sb[:, :],
                        in1=uu[:, :, :],
                        op0=AL.add,
                        op1=AL.add,
                    )
                    if g % 2 == 1:
                        rr = r0 + 4 * (g - 1)
                        nc.sync.dma_start(
                            out=o_d[:, rr : rr + 8, :],
                            in_=ob[:, 4 * (g - 1) : 4 * (g + 1), :],
                        )

    nc.compile()
    return nc


def _get_nc():
    if "nc" not in _CACHE:
        _CACHE["nc"] = _build()
    return _CACHE["nc"]


def _shard_inputs(x, weight, bias):
    x = np.asarray(x, dtype=np.float32)
    weight = np.asarray(weight, dtype=np.float32)
    bias = np.asarray(bias, dtype=np.float32)
    in_maps = []
    for c in range(N_CORES):
        sl = slice(SPC * c, SPC * (c + 1))
        xs = np.ascontiguousarray(x[sl]).reshape(128, H, W).astype(np.float16)
        # [s, co, ci, ky, kx] -> [s, ci, ky, kx, co]
        wt = weight[sl].transpose(0, 2, 3, 4, 1)
        g0, g1, g2 = wt[:, :, :, 0, :], wt[:, :, :, 1, :], wt[:, :, :, 2, :]
        # winograd F(2,3) weight transform along kx; U3 negated so the
        # odd-output bank accumulates -M3
        um = np.stack(
            [g0, (g0 + g1 + g2) * 0.5, (g0 - g1 + g2) * 0.5, -g2], axis=2
        )  # [s, ci, m, ky, co]
        um = um.reshape(SPC, 32, 12, 32)
        ws = np.zeros((128, 12, 128), dtype=np.float16)
        for s in range(SPC):
            ws[32 * s : 32 * (s + 1), :, 32 * s : 32 * (s + 1)] = um[s]
        ws = ws.reshape(128, 12 * 128)
        bs = np.ascontiguousarray(bias[sl]).reshape(128, 1)
        in_maps.append({"x": xs, "w": ws, "bias_v": bs})
    return in_maps


def run(x, weight, bias, trace=False):
    from concourse.bass_utils import run_bass_kernel_spmd

    nc = _get_nc()
    in_maps = _shard_inputs(x, weight, bias)
    res = run_bass_kernel_spmd(
        nc, in_maps, core_ids=list(range(N_CORES)), trace=trace
    )
    out = np.empty((B, C_OUT, H, W), dtype=np.float32)
    for c in range(N_CORES):
        out[SPC * c : SPC * (c + 1)] = (
            res.results[c]["out"].astype(np.float32).reshape(SPC, C_OUT, H, W)
        )
    return out, res


def kernel(x, weight, bias):
    out, _ = run(x, weight, bias, trace=False)
    return out
